# Initial kernel scaffold
#
"""Per-pixel adaptive (kernel-prediction) 5x5 conv on 8 trn2 cores.

out[b,c,y,x] = sum_{i,j} x_pad[b,c,y+i,x+j] * kernel[b,(c*5+i)*5+j,y,x]
with edge (replication) padding p=2.

Sharding: 8 cores = B(4) x C-halves(2).  The op is depthwise (output
channel c reads only input channel c), so slicing C needs no halo.
Per core: xpad (16,260,260) f32, kern (400,256,256) f16 -> out (16,256,256).

Device layout: 128 SBUF partitions = 16 channels x 8 row-groups; each
partition owns a 36-row x 260-col stripe of padded x (halo included), so
every tap (i,j) is a strided view at free offset i*260+j.  The kernel
tensor is converted to fp16 on the host to halve the dominant HBM
traffic (~105MB -> 52MB per core).

Per 16-row half-pass: DVE computes the 25 tap products (f32 x * f16 k),
and the otherwise-idle TensorE accumulates them into PSUM via identity
matmuls (PSUM accumulate-on-write does the adds for free).  ScalarE
drains PSUM to SBUF; gpsimd SWDGE stores to DRAM.
"""

import numpy as np

B, C, H, W, K = 4, 32, 256, 256, 5
P = (K - 1) // 2  # 2
CP = 16           # channels per core
YG = 8            # row groups
RG = H // YG      # 32 rows per group
WP = W + 2 * P    # 260
SROWS = RG + 2 * P  # 36 rows per stripe
SLEN = SROWS * WP   # 9360 elems per partition stripe
HR = RG // 2        # 16 rows per half-pass
HFREE = HR * W      # 4096 free elems per half-pass
NBANK = HFREE // 512  # 8 psum banks

_cache = {}


def _build_nc():
    import concourse.bass as bass
    import concourse.tile as tile
    from concourse import bacc, mybir

    f32 = mybir.dt.float32
    f16 = mybir.dt.float16
    nc = bacc.Bacc("TRN2", target_bir_lowering=False, debug=False, num_devices=8)

    xpad_t = nc.dram_tensor("xpad", [CP, WP, WP], f16, kind="ExternalInput")
    kern_t = nc.dram_tensor("kern", [CP * K * K, H, W], f16, kind="ExternalInput")
    ident_t = nc.dram_tensor("ident", [128, 128], f16, kind="ExternalInput")
    out_t = nc.dram_tensor("out", [CP, H, W], f16, kind="ExternalOutput")

    with tile.TileContext(nc) as tc:
        with (
            tc.tile_pool(name="xp", bufs=1) as xpool,
            tc.tile_pool(name="idp", bufs=1) as ipool,
            tc.tile_pool(name="kp", bufs=8) as kpool,
            tc.tile_pool(name="tp", bufs=3) as tpool,
            tc.tile_pool(name="op", bufs=2) as opool,
            tc.tile_pool(name="pp", bufs=1, space="PSUM") as ppool,
        ):
            xtile = xpool.tile([128, SLEN], f16)
            # partition (c,g) <- xpad[c, g*RG : g*RG+SROWS, :], contiguous.
            # Two chunks: rows 0..19 gate pass h=0, rows 20..35 only h=1.
            NA = 20 * WP
            srcA = bass.AP(xpad_t, 0, [[WP * WP, CP], [RG * WP, YG], [1, NA]])
            nc.gpsimd.dma_start(out=xtile[:, :NA], in_=srcA)
            srcB = bass.AP(
                xpad_t, NA, [[WP * WP, CP], [RG * WP, YG], [1, SLEN - NA]]
            )
            nc.gpsimd.dma_start(out=xtile[:, NA:], in_=srcB)

            ident = ipool.tile([128, 128], f16)
            nc.gpsimd.dma_start(out=ident[:], in_=ident_t[:, :])

            x3 = xtile[:].rearrange("p (r w) -> p r w", w=WP)

            for h in range(2):
                ptile = ppool.tile([128, HFREE], f32, tag="ps")
                for ij in range(K * K):
                    i, j = divmod(ij, K)
                    ktile = kpool.tile([128, HFREE], f16, tag="kt")
                    ksrc = bass.AP(
                        kern_t,
                        ij * H * W + h * HFREE,
                        [[K * K * H * W, CP], [RG * W, YG], [1, HFREE]],
                    )
                    keng = nc.sync if ij % 2 == 0 else nc.scalar
                    keng.dma_start(out=ktile[:], in_=ksrc, single_packet=True)
                    k3 = ktile[:].rearrange("p (r w) -> p r w", w=W)
                    r0 = h * HR
                    xv = x3[:, i + r0 : i + r0 + HR, j : j + W]
                    tmp = tpool.tile([128, HFREE], f16, tag="tmp")
                    t3 = tmp[:].rearrange("p (r w) -> p r w", w=W)
                    nc.vector.tensor_mul(t3, xv, k3)
                    for b in range(NBANK):
                        nc.tensor.matmul(
                            out=ptile[:, b * 512 : (b + 1) * 512],
                            lhsT=ident[:],
                            rhs=tmp[:, b * 512 : (b + 1) * 512],
                            start=(ij == 0),
                            stop=(ij == K * K - 1),
                        )
                HHALF = HFREE // 2
                obuf = opool.tile([128, HFREE], f16, tag="ob")
                for q in range(2):
                    nc.scalar.copy(
                        obuf[:, q * HHALF : (q + 1) * HHALF],
                        ptile[:, q * HHALF : (q + 1) * HHALF],
                    )
                    dst = bass.AP(
                        out_t,
                        h * HFREE + q * HHALF,
                        [[H * W, CP], [RG * W, YG], [1, HHALF]],
                    )
                    nc.gpsimd.dma_start(
                        out=dst, in_=obuf[:, q * HHALF : (q + 1) * HHALF]
                    )

    nc.compile()
    return nc


def _get_nc():
    if "nc" not in _cache:
        _cache["nc"] = _build_nc()
    return _cache["nc"]


_IDENT = np.eye(128, dtype=np.float16)


def _make_in_map(xpad, kern_bf16, b, c0):
    return {
        "xpad": np.ascontiguousarray(xpad[b, c0 : c0 + CP]),
        "kern": np.ascontiguousarray(kern_bf16[b, c0 * K * K : (c0 + CP) * K * K]),
        "ident": _IDENT,
    }


def kernel(x, kernel, kernel_size):
    from concourse.bass_utils import run_bass_kernel_spmd

    x = np.asarray(x, dtype=np.float32).astype(np.float16)
    kern = np.asarray(kernel, dtype=np.float32).astype(np.float16)
    xpad = np.pad(x, ((0, 0), (0, 0), (P, P), (P, P)), mode="edge")

    in_maps = []
    for core in range(8):
        b, half = divmod(core, 2)
        c0 = half * CP
        in_maps.append(_make_in_map(xpad, kern, b, c0))

    nc = _get_nc()
    res = run_bass_kernel_spmd(nc, in_maps, list(range(8)))

    out = np.empty((B, C, H, W), dtype=np.float32)
    for core in range(8):
        b, half = divmod(core, 2)
        c0 = half * CP
        out[b, c0 : c0 + CP] = res.results[core]["out"].astype(np.float32)
    return out



# revision 14
# speedup vs baseline: 2.0325x; 2.0325x over previous
"""Per-pixel adaptive (kernel-prediction) 5x5 conv on 8 trn2 cores.

out[b,c,y,x] = sum_{i,j} x_pad[b,c,y+i,x+j] * kernel[b,(c*5+i)*5+j,y,x]
with edge (replication) padding p=2.

Sharding: 8 cores = B(4) x C-halves(2).  The op is depthwise (output
channel c reads only input channel c), so slicing C needs no halo.

v2: the kernel tensor dominates HBM traffic and each SDMA engine moves
read+write bytes at ~27 GB/s combined, so the win is byte reduction on
BOTH sides of the DMA.  The kernel is quantized host-side to int8 with a
single global scale (folded into x, also host-side); per-core traffic
drops 52.4MB -> 26.2MB.  On-device the int8 tiles are cast to fp16 by
the otherwise-idle Scalar and GpSimd engines (plus a few on Vector), so
the DVE only does the 50 tap multiplies and TensorE accumulates them
into PSUM via identity matmuls as before.

Layout: 128 SBUF partitions = 16 channels x 8 row-groups; each partition
owns a 36-row x 260-col stripe of padded, pre-scaled x (fp16), so every
tap (i,j) is a strided view at free offset i*260+j.  A second stripe
copy shifted by one element keeps odd-j taps 4B-aligned for the DVE's
2x packed fp16 mode.  Taps are processed evens-first so the shifted
copy (built on GpSimd) is ready before any odd tap needs it.
"""

import numpy as np

B, C, H, W, K = 4, 32, 256, 256, 5
P = (K - 1) // 2  # 2
CP = 16           # channels per core
YG = 8            # row groups
RG = H // YG      # 32 rows per group
WP = W + 2 * P    # 260
SROWS = RG + 2 * P  # 36 rows per stripe
SLEN = SROWS * WP   # 9360 elems per partition stripe
HR = RG // 2        # 16 rows per half-pass
HFREE = HR * W      # 4096 free elems per half-pass
NBANK = HFREE // 512  # 8 psum banks
HW = H * W

# Tap order: evens (j in 0,2,4) first, odds (j in 1,3) after, paired into
# 2-tap DMA/cast chunks.  PSUM accumulation order is irrelevant.  Within
# every pair the element-offset delta between the two taps is constant
# (+2 or +256), so one 4-dim AP covers both taps and each pair runs as a
# single wide DVE op (halves DVE op count and inter-op semaphore waits).
_EVENS = [ij for ij in range(K * K) if (ij % K) % 2 == 0]   # 15 taps
_ODDS = [ij for ij in range(K * K) if (ij % K) % 2 == 1]    # 10 taps
_CHUNKS = (
    [tuple(_EVENS[k : k + 2]) for k in range(0, 14, 2)]  # 7 even pairs
    + [tuple(_ODDS[k : k + 2]) for k in range(0, 4, 2)]  # odd pairs 0,1
    + [tuple(_ODDS[k : k + 2]) for k in range(4, 10, 2)]  # odd pairs 2..4
    + [(_EVENS[14],)]                                     # 1 even single
)
# Per-chunk path: ACT casts int8->fp16 then DVE does a fp16 mul (2x
# packed mode; odd-j taps read the 1-elem-shifted x copy to stay
# 4B-aligned); 'stt' chunks run DVE scalar_tensor_tensor straight off
# the int8 tile (1x mode, fused cast+mul, alignment-immune).  GpSimd
# does no elementwise work at all: its ucode CAST/COPY measured 3-4x
# slower than modeled and its SBUF port contention slowed every DVE op
# ~2.5x.  ACT ~146us and DVE ~144us busy with this split.
_CAST_ENG = ["act"] * 9 + ["stt"] * 3 + ["act"]

_cache = {}


def _build_nc():
    import concourse.bass as bass
    import concourse.tile as tile
    from concourse import bacc, mybir

    f32 = mybir.dt.float32
    f16 = mybir.dt.float16
    i8 = mybir.dt.int8
    nc = bacc.Bacc("TRN2", target_bir_lowering=False, debug=False, num_devices=8)

    xpad_t = nc.dram_tensor("xpad", [CP, WP, WP], f16, kind="ExternalInput")
    kern_t = nc.dram_tensor("kern", [CP * K * K, H, W], i8, kind="ExternalInput")
    ident_t = nc.dram_tensor("ident", [128, 128], f16, kind="ExternalInput")
    out_t = nc.dram_tensor("out", [CP, H, W], f16, kind="ExternalOutput")

    with tile.TileContext(nc) as tc:
        with (
            tc.tile_pool(name="xp", bufs=1) as xpool,
            tc.tile_pool(name="idp", bufs=1) as ipool,
            tc.tile_pool(name="kq", bufs=5) as kqpool,
            tc.tile_pool(name="kf", bufs=3) as kfpool,
            tc.tile_pool(name="tp", bufs=4) as tpool,
            tc.tile_pool(name="op", bufs=2) as opool,
            tc.tile_pool(name="pp", bufs=1, space="PSUM") as ppool,
        ):
            xtile = xpool.tile([128, SLEN], f16)
            # partition (c,g) <- xpad[c, g*RG : g*RG+SROWS, :], contiguous.
            # Two chunks: rows 0..19 gate pass h=0, rows 20..35 only h=1.
            NA = 20 * WP
            srcA = bass.AP(xpad_t, 0, [[WP * WP, CP], [RG * WP, YG], [1, NA]])
            nc.gpsimd.dma_start(out=xtile[:, :NA], in_=srcA)
            srcB = bass.AP(
                xpad_t, NA, [[WP * WP, CP], [RG * WP, YG], [1, SLEN - NA]]
            )
            nc.gpsimd.dma_start(out=xtile[:, NA:], in_=srcB)

            ident = ipool.tile([128, 128], f16)
            nc.gpsimd.dma_start(out=ident[:], in_=ident_t[:, :])

            x3 = xtile[:].rearrange("p (r w) -> p r w", w=WP)
            xo3 = None
            xodd = None

            for h in range(2):
                ptile = ppool.tile([128, HFREE], f32, tag="ps")
                r0 = h * HR
                for ci, ts in enumerate(_CHUNKS):
                    nt = len(ts)
                    kq = kqpool.tile([128, nt * HFREE], i8, tag="kq")
                    for s, ij in enumerate(ts):
                        ksrc = bass.AP(
                            kern_t,
                            ij * HW + h * HFREE,
                            [[K * K * HW, CP], [RG * W, YG], [1, HFREE]],
                        )
                        nc.sync.dma_start(
                            out=kq[:, s * HFREE : (s + 1) * HFREE],
                            in_=ksrc,
                            single_packet=True,
                        )

                    if h == 0 and ci == 7:
                        # Shifted stripe so odd-j taps read at even element
                        # offsets (4B-aligned -> DVE 2x packed mode).  Built
                        # on DVE (2x_2p single-src mode, ~5us); emitted here
                        # so the even chunks' muls run first.
                        xodd = xpool.tile([128, SLEN], f16)
                        nc.vector.tensor_copy(xodd[:, : SLEN - 1], xtile[:, 1:])
                        xo3 = xodd[:].rearrange("p (r w) -> p r w", w=WP)

                    eng = _CAST_ENG[ci]
                    if eng == "act":
                        kf = kfpool.tile([128, nt * HFREE], f16, tag="kf")
                        nc.scalar.copy(kf[:], kq[:])

                    for s, ij in enumerate(ts):
                        i, j = divmod(ij, K)
                        tmp = tpool.tile([128, HFREE], f16, tag="tmp")
                        t3 = tmp[:].rearrange("p (r w) -> p r w", w=W)
                        if eng == "act":
                            if j % 2 == 0:
                                xv = x3[:, i + r0 : i + r0 + HR, j : j + W]
                            else:
                                xv = xo3[:, i + r0 : i + r0 + HR, j - 1 : j - 1 + W]
                            k3 = kf[:, s * HFREE : (s + 1) * HFREE].rearrange(
                                "p (r w) -> p r w", w=W
                            )
                            nc.vector.tensor_mul(t3, xv, k3)
                        else:
                            # fused int8 cast + multiply on DVE (1x mode)
                            xv = x3[:, i + r0 : i + r0 + HR, j : j + W]
                            q3 = kq[:, s * HFREE : (s + 1) * HFREE].rearrange(
                                "p (r w) -> p r w", w=W
                            )
                            nc.vector.scalar_tensor_tensor(
                                t3, q3, 1.0, xv,
                                op0=mybir.AluOpType.mult,
                                op1=mybir.AluOpType.mult,
                            )
                        first = ci == 0 and s == 0
                        last = ci == len(_CHUNKS) - 1 and s == nt - 1
                        for b in range(NBANK):
                            nc.tensor.matmul(
                                out=ptile[:, b * 512 : (b + 1) * 512],
                                lhsT=ident[:],
                                rhs=tmp[:, b * 512 : (b + 1) * 512],
                                start=first,
                                stop=last,
                            )
                HHALF = HFREE // 2
                obuf = opool.tile([128, HFREE], f16, tag="ob")
                for q in range(2):
                    nc.scalar.copy(
                        obuf[:, q * HHALF : (q + 1) * HHALF],
                        ptile[:, q * HHALF : (q + 1) * HHALF],
                    )
                    dst = bass.AP(
                        out_t,
                        h * HFREE + q * HHALF,
                        [[H * W, CP], [RG * W, YG], [1, HHALF]],
                    )
                    nc.gpsimd.dma_start(
                        out=dst, in_=obuf[:, q * HHALF : (q + 1) * HHALF]
                    )

    nc.compile()
    return nc


def _get_nc():
    if "nc" not in _cache:
        _cache["nc"] = _build_nc()
    return _cache["nc"]


_IDENT = np.eye(128, dtype=np.float16)


def prepare_in_maps(x, kern):
    """Host-side prep: int8-quantize kern (global scale folded into x),
    edge-pad + scale x to fp16, slice per core."""
    x = np.asarray(x, dtype=np.float32)
    kern = np.asarray(kern, dtype=np.float32)
    delta = float(np.abs(kern).max()) / 127.0
    q = np.clip(np.round(kern / delta), -127, 127).astype(np.int8)
    xpad = np.pad(x, ((0, 0), (0, 0), (P, P), (P, P)), mode="edge")
    xs = (xpad * delta).astype(np.float16)

    in_maps = []
    for core in range(8):
        b, half = divmod(core, 2)
        c0 = half * CP
        in_maps.append({
            "xpad": np.ascontiguousarray(xs[b, c0 : c0 + CP]),
            "kern": np.ascontiguousarray(
                q[b, c0 * K * K : (c0 + CP) * K * K]
            ),
            "ident": _IDENT,
        })
    return in_maps


def kernel(x, kernel, kernel_size):
    from concourse.bass_utils import run_bass_kernel_spmd

    in_maps = prepare_in_maps(x, kernel)
    nc = _get_nc()
    res = run_bass_kernel_spmd(nc, in_maps, list(range(8)))

    out = np.empty((B, C, H, W), dtype=np.float32)
    for core in range(8):
        b, half = divmod(core, 2)
        c0 = half * CP
        out[b, c0 : c0 + CP] = res.results[core]["out"].astype(np.float32)
    return out


# revision 24
# speedup vs baseline: 2.3892x; 1.1755x over previous
"""Per-pixel adaptive (kernel-prediction) 5x5 conv on 8 trn2 cores.

out[b,c,y,x] = sum_{i,j} x_pad[b,c,y+i,x+j] * kernel[b,(c*5+i)*5+j,y,x]
with edge (replication) padding p=2.

Sharding: 8 cores = B(4) x C-halves(2).  The op is depthwise (output
channel c reads only input channel c), so slicing C needs no halo.

v2: the kernel tensor dominates HBM traffic and each SDMA engine moves
read+write bytes at ~27 GB/s combined, so the win is byte reduction on
BOTH sides of the DMA.  The kernel is quantized host-side to int8 with a
single global scale (folded into x, also host-side); per-core traffic
drops 52.4MB -> 26.2MB.  On-device the int8 tiles are cast to fp16 by
the otherwise-idle Scalar and GpSimd engines (plus a few on Vector), so
the DVE only does the 50 tap multiplies and TensorE accumulates them
into PSUM via identity matmuls as before.

Layout: 128 SBUF partitions = 16 channels x 8 row-groups; each partition
owns a 36-row x 260-col stripe of padded, pre-scaled x (fp16), so every
tap (i,j) is a strided view at free offset i*260+j.  A second stripe
copy shifted by one element keeps odd-j taps 4B-aligned for the DVE's
2x packed fp16 mode.  Taps are processed evens-first so the shifted
copy (built on GpSimd) is ready before any odd tap needs it.
"""

import numpy as np

B, C, H, W, K = 4, 32, 256, 256, 5
P = (K - 1) // 2  # 2
CP = 16           # channels per core
YG = 8            # row groups
RG = H // YG      # 32 rows per group
WP = W + 2 * P    # 260
SROWS = RG + 2 * P  # 36 rows per stripe
SLEN = SROWS * WP   # 9360 elems per partition stripe
HR = RG // 2        # 16 rows per half-pass
HFREE = HR * W      # 4096 free elems per half-pass
NBANK = HFREE // 512  # 8 psum banks
HW = H * W

# Tap order: evens (j in 0,2,4) first, odds (j in 1,3) after, paired into
# 2-tap DMA/cast chunks.  PSUM accumulation order is irrelevant.  Within
# every pair the element-offset delta between the two taps is constant
# (+2 or +256), so one 4-dim AP covers both taps and each pair runs as a
# single wide DVE op (halves DVE op count and inter-op semaphore waits).
_EVENS = [ij for ij in range(K * K) if (ij % K) % 2 == 0]   # 15 taps
_ODDS = [ij for ij in range(K * K) if (ij % K) % 2 == 1]    # 10 taps
_EP = [tuple(_EVENS[k : k + 2]) for k in range(0, 14, 2)]   # 7 even pairs
_OP = [tuple(_ODDS[k : k + 2]) for k in range(0, 10, 2)]    # 5 odd pairs
# Per-chunk path:
#  'act'  - ScalarE casts int8->fp16 (on-model 1.74ns/elem), DVE fp16 mul
#           in 2x packed mode (odd-j taps read the 1-elem-shifted x copy
#           to stay 4B-aligned).
#  'dma'  - SWDGE casting DMA loads fp16 directly (int8 HBM read, fp16
#           SBUF write); zero engine cost, DVE fp16 mul.  Uses spare DMA
#           beat capacity to offload ScalarE.
#  'stt'  - DVE scalar_tensor_tensor straight off the int8 tile (1x
#           mode, fused cast+mul, alignment-immune).
# GpSimd runs no elementwise work: its ucode CAST/COPY measured 3-4x
# slower than modeled and its SBUF port contention slowed every DVE op
# ~2.5x.  dma/stt chunks are interleaved between act chunks so ScalarE
# casts and DVE muls stream concurrently; each half ends on a 'dma'
# chunk whose tail chain (mul+matmul only) is shortest.
_SEQ = [
    ((_EVENS[14],), "act"),  # single first: fastest first cast
    (_EP[0], "act"),
    (_EP[1], "act"),
    (_EP[2], "act"),         # xodd is built here, before any odd chunk
    (_OP[0], "act"),
    (_EP[3], "act"),
    (_OP[1], "stt"),
    (_EP[4], "act"),
    (_OP[2], "dma"),
    (_EP[5], "act"),
    (_EP[6], "act"),
    (_OP[3], "act"),
    (_OP[4], "dma"),
]

_cache = {}


def _build_nc():
    import concourse.bass as bass
    import concourse.tile as tile
    from concourse import bacc, mybir

    f32 = mybir.dt.float32
    f16 = mybir.dt.float16
    i8 = mybir.dt.int8
    nc = bacc.Bacc("TRN2", target_bir_lowering=False, debug=False, num_devices=8)

    xpad_t = nc.dram_tensor("xpad", [CP, WP, WP], f16, kind="ExternalInput")
    kern_t = nc.dram_tensor("kern", [CP * K * K, H, W], i8, kind="ExternalInput")
    ident_t = nc.dram_tensor("ident", [128, 128], f16, kind="ExternalInput")
    out_t = nc.dram_tensor("out", [CP, H, W], f16, kind="ExternalOutput")

    with tile.TileContext(nc) as tc:
        with (
            tc.tile_pool(name="xp", bufs=1) as xpool,
            tc.tile_pool(name="idp", bufs=1) as ipool,
            tc.tile_pool(name="kq", bufs=4) as kqpool,
            tc.tile_pool(name="kf", bufs=4) as kfpool,
            tc.tile_pool(name="tp", bufs=4) as tpool,
            tc.tile_pool(name="op", bufs=2) as opool,
            tc.tile_pool(name="pp", bufs=1, space="PSUM") as ppool,
        ):
            # Warm the ScalarE activation table before any data arrives so
            # the first real cast doesn't pay the lazy ACT_TABLE_LOAD.
            dwarm = ipool.tile([128, 2], f16)
            nc.gpsimd.memset(dwarm[:], 0.0)
            dwarm2 = ipool.tile([128, 2], f16)
            nc.scalar.copy(dwarm2[:], dwarm[:])

            xtile = xpool.tile([128, SLEN], f16)
            ident = ipool.tile([128, 128], f16)
            nc.gpsimd.dma_start(out=ident[:], in_=ident_t[:, :])

            x3 = xtile[:].rearrange("p (r w) -> p r w", w=WP)
            xo3 = None
            xodd = None

            def ksrc_ap(ij, h):
                return bass.AP(
                    kern_t,
                    ij * HW + h * HFREE,
                    [[K * K * HW, CP], [RG * W, YG], [1, HFREE]],
                )

            for h in range(2):
                ptile = ppool.tile([128, HFREE], f32, tag="ps")
                r0 = h * HR
                for ci, (ts, eng) in enumerate(_SEQ):
                    nt = len(ts)
                    if eng == "dma":
                        # SWDGE casting load: int8 HBM -> fp16 SBUF
                        kf = kfpool.tile([128, nt * HFREE], f16, tag="kf")
                        for s, ij in enumerate(ts):
                            nc.gpsimd.dma_start(
                                out=kf[:, s * HFREE : (s + 1) * HFREE],
                                in_=ksrc_ap(ij, h),
                                single_packet=True,
                            )
                        kq = None
                    else:
                        kq = kqpool.tile([128, nt * HFREE], i8, tag="kq")
                        for s, ij in enumerate(ts):
                            nc.sync.dma_start(
                                out=kq[:, s * HFREE : (s + 1) * HFREE],
                                in_=ksrc_ap(ij, h),
                                single_packet=True,
                            )

                    if h == 0 and ci == 0:
                        # x stripes ride the same HWDGE ring as the kernel
                        # chunks, AFTER chunk 0: strict FIFO gives chunk 0
                        # the full early DMA rate, and x still lands well
                        # before the first multiply needs it.
                        # partition (c,g) <- xpad[c, g*RG:g*RG+SROWS, :].
                        NA = 20 * WP
                        srcA = bass.AP(
                            xpad_t, 0, [[WP * WP, CP], [RG * WP, YG], [1, NA]]
                        )
                        nc.sync.dma_start(out=xtile[:, :NA], in_=srcA)
                        srcB = bass.AP(
                            xpad_t,
                            NA,
                            [[WP * WP, CP], [RG * WP, YG], [1, SLEN - NA]],
                        )
                        nc.sync.dma_start(out=xtile[:, NA:], in_=srcB)

                    if h == 0 and ci == 2:
                        # Shifted stripe so odd-j taps read at even element
                        # offsets (4B-aligned -> DVE 2x packed mode).  Built
                        # on DVE (2x_2p single-src mode, ~5us); emitted here
                        # so the even chunks' muls run first.
                        xodd = xpool.tile([128, SLEN], f16)
                        nc.vector.tensor_copy(xodd[:, : SLEN - 1], xtile[:, 1:])
                        xo3 = xodd[:].rearrange("p (r w) -> p r w", w=WP)

                    if eng == "act":
                        kf = kfpool.tile([128, nt * HFREE], f16, tag="kf")
                        nc.scalar.copy(kf[:], kq[:])

                    tmps = []
                    for s, ij in enumerate(ts):
                        i, j = divmod(ij, K)
                        tmp = tpool.tile([128, HFREE], f16, tag="tmp")
                        t3 = tmp[:].rearrange("p (r w) -> p r w", w=W)
                        if eng in ("act", "dma"):
                            if j % 2 == 0:
                                xv = x3[:, i + r0 : i + r0 + HR, j : j + W]
                            else:
                                xv = xo3[:, i + r0 : i + r0 + HR, j - 1 : j - 1 + W]
                            k3 = kf[:, s * HFREE : (s + 1) * HFREE].rearrange(
                                "p (r w) -> p r w", w=W
                            )
                            nc.vector.tensor_mul(t3, xv, k3)
                        else:
                            # fused int8 cast + multiply on DVE (1x mode)
                            xv = x3[:, i + r0 : i + r0 + HR, j : j + W]
                            q3 = kq[:, s * HFREE : (s + 1) * HFREE].rearrange(
                                "p (r w) -> p r w", w=W
                            )
                            nc.vector.scalar_tensor_tensor(
                                t3, q3, 1.0, xv,
                                op0=mybir.AluOpType.mult,
                                op1=mybir.AluOpType.mult,
                            )
                        if ci < len(_SEQ) - 1:
                            for b in range(NBANK):
                                nc.tensor.matmul(
                                    out=ptile[:, b * 512 : (b + 1) * 512],
                                    lhsT=ident[:],
                                    rhs=tmp[:, b * 512 : (b + 1) * 512],
                                    start=ci == 0 and s == 0,
                                    stop=False,
                                )
                        else:
                            tmps.append(tmp)
                    if ci == len(_SEQ) - 1:
                        # Bank-major for the final chunk: low banks retire
                        # first so their drains overlap the remaining mms.
                        for b in range(NBANK):
                            for s, tmp in enumerate(tmps):
                                nc.tensor.matmul(
                                    out=ptile[:, b * 512 : (b + 1) * 512],
                                    lhsT=ident[:],
                                    rhs=tmp[:, b * 512 : (b + 1) * 512],
                                    start=False,
                                    stop=s == len(tmps) - 1,
                                )
                QD = HFREE // 4
                obuf = opool.tile([128, HFREE], f16, tag="ob")
                for q in range(4):
                    nc.scalar.copy(
                        obuf[:, q * QD : (q + 1) * QD],
                        ptile[:, q * QD : (q + 1) * QD],
                    )
                    dst = bass.AP(
                        out_t,
                        h * HFREE + q * QD,
                        [[H * W, CP], [RG * W, YG], [1, QD]],
                    )
                    nc.gpsimd.dma_start(
                        out=dst, in_=obuf[:, q * QD : (q + 1) * QD]
                    )

    nc.compile()
    return nc


def _get_nc():
    if "nc" not in _cache:
        _cache["nc"] = _build_nc()
    return _cache["nc"]


_IDENT = np.eye(128, dtype=np.float16)


def prepare_in_maps(x, kern):
    """Host-side prep: int8-quantize kern (global scale folded into x),
    edge-pad + scale x to fp16, slice per core."""
    x = np.asarray(x, dtype=np.float32)
    kern = np.asarray(kern, dtype=np.float32)
    delta = float(np.abs(kern).max()) / 127.0
    q = np.clip(np.round(kern / delta), -127, 127).astype(np.int8)
    xpad = np.pad(x, ((0, 0), (0, 0), (P, P), (P, P)), mode="edge")
    xs = (xpad * delta).astype(np.float16)

    in_maps = []
    for core in range(8):
        b, half = divmod(core, 2)
        c0 = half * CP
        in_maps.append({
            "xpad": np.ascontiguousarray(xs[b, c0 : c0 + CP]),
            "kern": np.ascontiguousarray(
                q[b, c0 * K * K : (c0 + CP) * K * K]
            ),
            "ident": _IDENT,
        })
    return in_maps


def kernel(x, kernel, kernel_size):
    from concourse.bass_utils import run_bass_kernel_spmd

    in_maps = prepare_in_maps(x, kernel)
    nc = _get_nc()
    res = run_bass_kernel_spmd(nc, in_maps, list(range(8)))

    out = np.empty((B, C, H, W), dtype=np.float32)
    for core in range(8):
        b, half = divmod(core, 2)
        c0 = half * CP
        out[b, c0 : c0 + CP] = res.results[core]["out"].astype(np.float32)
    return out
